# revision 33
# baseline (speedup 1.0000x reference)
"""Trainium2 Bass kernel for nn_HardestContrastiveLoss.

Strategy (1D row-parallel cdist, per sharding hint):
  - Host: gather the selected correspondences (pure indexing/layout), build
    transposed operand blocks with constant padding rows, shard 8192 selected
    rows as 1024 rows/core.
  - Device (per core, identical program, different data):
      * prep: rigid-transform gathered src points, square feats/points; all
        norm and threshold terms ride as extra contraction rows of the main
        GEMMs (matmul cost is streaming-bound, independent of K):
        psf  = [-2sf | sf^2 | 1] . [tf | 1 | tf^2]        K=96 -> feats_d2
        pneg = [2S sp'|sp'^2|SC2|-S] . [tp|-S|1|tp^2]     K=10 -> -S(pts_d2-C2)
        ppos = same with SC1                              K=10 -> -S(pts_d2-C1)
      * hardest-positive per row:  max_j min(psf, ppos)   (clamped at 0)
        closest-negative per row:  min_j max(psf, pneg)   (init BIG)
        ONE fused DVE tensor_tensor_reduce per (tile, path) for most n-tiles;
        the rest accumulate elementwise on the Pool engine (Pool cannot
        row-reduce) with one DVE reduce per row-block.
      * all matmuls in float32r (1 PE cycle/row at free-size 512 vs 4 for
        fp32); sqrt/thresholds deferred to the [128, 8] tail.
  - Host: sum the 8 per-core [2,1] partials, divide by N (the "all-reduce").
"""

import numpy as np

N_SEL = 8192
N_CORES = 8
ROWS_PER_CORE = N_SEL // N_CORES  # 1024
M_TILES = ROWS_PER_CORE // 128  # 8
NT = 512
N_TILES = N_SEL // NT  # 16
K_F = 96  # feats ext contraction: 32 feats + 32 row-norm + 32 col-norm
K_P = 10  # pts ext contraction: 3 pts + 3 row-norm + 1 const + 3 col-norm

EPS = 1e-7
POS_RADIUS = 0.0375
NEG_RADIUS = 0.1
POS_THRESH = 0.1
NEG_THRESH = 1.4
C1 = float(np.float32(POS_RADIUS**2 - EPS))  # pos: pd2 < C1
C2 = float(np.float32(NEG_RADIUS**2 - EPS))  # neg: pd2 > C2
S = 1.0e13
BIG = 100000.0

_PROGRAM_CACHE: dict = {}
KERNEL_CFG = {"dve_tiles": 16, "mm": "f32r", "wide": True}


def _register_fused_ops():
    """Register two custom DVE ops (select + row-reduce fused in one
    instruction): pos = max-reduce of min(in0, in1), neg = min-reduce of
    max(in0, in1); accum seeded from s0. Idempotent; appended after the
    stock OPS so existing rows are untouched."""
    import re
    from concourse import dve_ops
    from concourse.dve_spec import Spec, Src0, Src1, C0, maxx, minn

    def _ref(ew_fn, red_fn, seed_fn):
        def _r(in0, in1, c0, c1, c2):
            b = ew_fn(in0.astype(np.float32), in1.astype(np.float32))
            r = red_fn(b.reshape(b.shape[0], -1), axis=-1, keepdims=True)
            return b, seed_fn(np.float32(c0), r)
        return _r

    refs = {
        "SELMIN_REDMAX_ANT": _ref(np.minimum, np.max, np.maximum),
        "SELMAX_REDMIN_ANT": _ref(np.maximum, np.min, np.minimum),
    }

    def reg(name, body, accum):
        if name in dve_ops._SUB_OPCODE_FOR_NAME:
            for op in dve_ops.OPS:
                if op.name == name:
                    return op
        op = dve_ops.DveOp(name, Spec(body=body, accum=accum, accum_init=C0,
                                      reference=refs[name]),
                           subdim=False, uops_sha={})
        dve_ops.OPS.append(op)
        dve_ops.CUSTOM_DVE_SPECS[name] = op.spec
        dve_ops._SUB_OPCODE_FOR_NAME[name] = (
            dve_ops._CUSTOM_DVE_ROW_BASE + len(dve_ops.OPS) - 1)
        for ver in ("v3", "v4"):
            try:
                op.compile(ver)
            except ValueError as e:
                m = re.search(r'"' + ver + r'"\]="([0-9a-f]+)"', str(e))
                if m:
                    op.uops_sha[ver] = m.group(1)
                    op.compile(ver)
        return op

    pos = reg("SELMIN_REDMAX_ANT", minn(Src0, Src1), maxx)
    neg = reg("SELMAX_REDMIN_ANT", maxx(Src0, Src1), minn)
    return pos, neg


def build_program(repeat: int = 1, dve_tiles: int = 16, mm: str = "f32r",
                  fused: bool = True, wide: bool = False, **_legacy):
    """Build the Bass program (one NeuronCore, run SPMD on 8).

    dve_tiles: how many of the 16 n-tiles use the fused DVE
    tensor_tensor_reduce for both paths; the rest run elementwise min/max
    accumulation on the Pool engine. mm: matmul input dtype ("f32r"|"f32").
    """
    import concourse.bacc as bacc
    import concourse.mybir as mybir
    import concourse.tile as tile

    f32 = mybir.dt.float32
    f32r = mybir.dt.float32r
    A = mybir.AluOpType
    AF = mybir.ActivationFunctionType
    X = mybir.AxisListType.X

    mmdt = f32r if mm == "f32r" else f32

    def ecast(ap):
        # engine-side (non-PE) view of an fp32r tile
        return ap.bitcast(f32) if mm == "f32r" else ap

    POS_OP, NEG_OP = _register_fused_ops()

    # n-tile -> engine assignment: interleave pool tiles among dve tiles so
    # both engines stay fed. pool_set = odd tiles first.
    n_pool = N_TILES - dve_tiles
    pool_set = set()
    n = 1
    while len(pool_set) < n_pool:
        pool_set.add(n)
        n = n + 2 if n + 2 < N_TILES else (0 if 0 not in pool_set else n + 1)
    pool_order = sorted(pool_set)

    nc = bacc.Bacc("TRN2", target_bir_lowering=False, debug=False,
                   num_devices=N_CORES)
    srcF_d = nc.dram_tensor("srcF", [K_F, ROWS_PER_CORE], mmdt,
                            kind="ExternalInput").ap()
    srcN_d = nc.dram_tensor("srcN", [K_P, ROWS_PER_CORE], mmdt,
                            kind="ExternalInput").ap()
    srcO_d = nc.dram_tensor("srcO", [K_P, ROWS_PER_CORE], mmdt,
                            kind="ExternalInput").ap()
    tgtF_d = nc.dram_tensor("tgtF", [64, N_SEL], mmdt,
                            kind="ExternalInput").ap()
    tgtP_d = nc.dram_tensor("tgtP", [K_P, N_SEL], mmdt,
                            kind="ExternalInput").ap()
    rtt_d = nc.dram_tensor("rtt", [3, 4], mmdt, kind="ExternalInput").ap()
    out_d = nc.dram_tensor("out", [2, 1], f32, kind="ExternalOutput").ap()

    with tile.TileContext(nc) as tc:
        with (
            tc.tile_pool(name="big", bufs=1) as big,
            tc.tile_pool(name="scr", bufs=2) as scr_p,
            tc.tile_pool(name="pscr", bufs=2) as pscr_p,
            tc.tile_pool(name="acc", bufs=2) as acc_p,
            tc.tile_pool(name="red", bufs=2) as red_p,
            tc.tile_pool(name="small", bufs=4) as small,
            tc.tile_pool(name="pf", bufs=2, space="PSUM") as pf_p,
            tc.tile_pool(name="pn", bufs=(1 if wide else 2),
                         space="PSUM") as pn_p,
            tc.tile_pool(name="pp", bufs=(1 if wide else 2),
                         space="PSUM") as pp_p,
        ):
            rhsF = big.tile([K_F, N_SEL], mmdt, tag="rhsF")
            rhsP = big.tile([K_P, N_SEL], mmdt, tag="rhsP")
            lhsF = big.tile([K_F, ROWS_PER_CORE], mmdt, tag="lhsF")
            lhsN = big.tile([K_P, ROWS_PER_CORE], mmdt, tag="lhsN")
            lhsP = big.tile([K_P, ROWS_PER_CORE], mmdt, tag="lhsP")
            rtt_sb = big.tile([3, 4], mmdt, tag="rtt")
            praw = big.tile([3, ROWS_PER_CORE], mmdt, tag="praw")
            prot = big.tile([3, ROWS_PER_CORE], f32, tag="prot")
            sq32 = big.tile([32, ROWS_PER_CORE], mmdt, tag="sq32")
            sqSP = big.tile([3, ROWS_PER_CORE], mmdt, tag="sqSP")
            sqt32 = big.tile([32, N_SEL], mmdt, tag="sqt32")
            sqP = big.tile([3, N_SEL], mmdt, tag="sqP")
            ones128 = big.tile([128, 1], f32, tag="ones128")
            fp2all = big.tile([128, M_TILES], f32, tag="fp2all")
            cn2all = big.tile([128, M_TILES], f32, tag="cn2all")
            accT = big.tile([128, 2], f32, tag="accT")
            beps = big.tile([128, 1], f32, tag="beps")
            bpos = big.tile([128, 1], f32, tag="bpos")
            bneg = big.tile([128, 1], f32, tag="bneg")

            nc.sync.dma_start(rhsF[0:64, :], tgtF_d[:])
            nc.sync.dma_start(rhsP[:], tgtP_d[:])
            nc.sync.dma_start(lhsF[:], srcF_d[:])
            nc.sync.dma_start(lhsN[:], srcN_d[:])
            nc.sync.dma_start(lhsP[:], srcO_d[:])
            nc.sync.dma_start(rtt_sb[:], rtt_d[:])
            nc.sync.dma_start(praw[:], srcN_d[0:3, :])
            nc.gpsimd.memset(ones128[:], 1.0)
            nc.gpsimd.memset(beps[:], EPS)
            nc.gpsimd.memset(bpos[:], -POS_THRESH)
            nc.gpsimd.memset(bneg[:], NEG_THRESH)

            # ---- squares of raw feats (before -2 scaling) ----
            nc.scalar.activation(sq32[:], ecast(lhsF[0:32, :]), AF.Square)
            nc.scalar.activation(sqt32[:], ecast(rhsF[0:32, :]), AF.Square)
            nc.scalar.activation(sqP[:], ecast(rhsP[0:3, :]), AF.Square)
            nc.sync.dma_start(lhsF[32:64, :], sq32[:])
            nc.sync.dma_start(rhsF[64:96, :], sqt32[:])
            nc.sync.dma_start(rhsP[7:10, :], sqP[:])
            nc.scalar.activation(lhsF[0:32, :], ecast(lhsF[0:32, :]),
                                 AF.Copy, bias=0.0, scale=-2.0)

            # ---- rigid transform: prot = R @ sp + t ----
            for ch in range(ROWS_PER_CORE // NT):
                sl = slice(ch * NT, (ch + 1) * NT)
                pch = pf_p.tile([3, NT], f32, tag="pf")
                nc.tensor.matmul(out=pch[:], lhsT=rtt_sb[0:3, 0:3],
                                 rhs=praw[:, sl], start=True,
                                 stop=True)
                nc.vector.tensor_scalar(out=prot[:, sl], in0=pch[:],
                                        scalar1=ecast(rtt_sb[0:3, 3:4]),
                                        scalar2=None, op0=A.add)
            nc.scalar.activation(sqSP[:], prot[:], AF.Square)
            nc.scalar.activation(lhsN[0:3, :], prot[:], AF.Copy,
                                 bias=0.0, scale=2.0 * S)
            nc.scalar.activation(lhsP[0:3, :], prot[:], AF.Copy,
                                 bias=0.0, scale=2.0 * S)
            nc.sync.dma_start(lhsN[3:6, :], sqSP[:])
            nc.sync.dma_start(lhsP[3:6, :], sqSP[:])

            WNT = 2 * NT if wide else NT
            WN_TILES = N_SEL // WNT
            pos_rA = big.tile([128, M_TILES * WN_TILES], f32, tag="pos_rA")
            neg_rA = big.tile([128, M_TILES * WN_TILES], f32, tag="neg_rA")

            def main_loop(_iv=None):
                for m in range(M_TILES):
                    msl = slice(m * 128, (m + 1) * 128)
                    pos_r = pos_rA[:, m * WN_TILES:(m + 1) * WN_TILES]
                    neg_r = neg_rA[:, m * WN_TILES:(m + 1) * WN_TILES]
                    for n in range(WN_TILES):
                        psf = pf_p.tile([128, WNT], f32, tag="pf")
                        png = pn_p.tile([128, WNT], f32, tag="pn")
                        pps = pp_p.tile([128, WNT], f32, tag="pp")
                        for g in range(WNT // NT):
                            nsl = slice(n * WNT + g * NT,
                                        n * WNT + (g + 1) * NT)
                            gsl = slice(g * NT, (g + 1) * NT)
                            nc.tensor.matmul(out=psf[:, gsl],
                                             lhsT=lhsF[:, msl],
                                             rhs=rhsF[:, nsl],
                                             start=True, stop=True)
                            nc.tensor.matmul(out=png[:, gsl],
                                             lhsT=lhsN[:, msl],
                                             rhs=rhsP[:, nsl],
                                             start=True, stop=True)
                            nc.tensor.matmul(out=pps[:, gsl],
                                             lhsT=lhsP[:, msl],
                                             rhs=rhsP[:, nsl],
                                             start=True, stop=True)
                        fsb = scr_p.tile([128, WNT], f32, tag="fsb")
                        nc.scalar.copy(fsb[:], psf[:])
                        s1 = scr_p.tile([128, WNT], f32, tag="s1")
                        s2 = scr_p.tile([128, WNT], f32, tag="s2")
                        if fused:
                            nc.vector._custom_dve(
                                POS_OP, out=s1[:], in0=fsb[:], in1=pps[:],
                                s0=0.0, accum_out=pos_r[:, n:n + 1])
                            nc.vector._custom_dve(
                                NEG_OP, out=s2[:], in0=fsb[:], in1=png[:],
                                s0=BIG, accum_out=neg_r[:, n:n + 1])
                        else:
                            nc.vector.tensor_tensor(out=s1[:], in0=fsb[:],
                                                    in1=pps[:], op=A.min)
                            nc.vector.tensor_tensor(out=s2[:], in0=fsb[:],
                                                    in1=png[:], op=A.max)
                            nc.vector.tensor_reduce(
                                out=pos_r[:, n:n + 1], in_=s1[:], op=A.max,
                                axis=X)
                            nc.vector.tensor_reduce(
                                out=neg_r[:, n:n + 1], in_=s2[:], op=A.min,
                                axis=X)
            if repeat <= 4:
                for _ in range(repeat):
                    main_loop()
            else:
                with tc.For_i(0, repeat, 1) as iv:
                    main_loop(iv)

            # ---- tail: clamp / sqrt / relu thresholds / partition sums ----
            for m in range(M_TILES):
                nc.vector.tensor_reduce(
                    out=fp2all[:, m:m + 1],
                    in_=pos_rA[:, m * WN_TILES:(m + 1) * WN_TILES],
                    op=A.max, axis=X)
                nc.vector.tensor_reduce(
                    out=cn2all[:, m:m + 1],
                    in_=neg_rA[:, m * WN_TILES:(m + 1) * WN_TILES],
                    op=A.min, axis=X)
            fpc = small.tile([128, M_TILES], f32, tag="fpc")
            cnc = small.tile([128, M_TILES], f32, tag="cnc")
            nc.scalar.activation(fpc[:], fp2all[:], AF.Relu)
            nc.scalar.activation(cnc[:], cn2all[:], AF.Relu)
            fp = small.tile([128, M_TILES], f32, tag="fp")
            cn = small.tile([128, M_TILES], f32, tag="cn")
            nc.scalar.activation(fp[:], fpc[:], AF.Sqrt, bias=beps[:])
            nc.scalar.activation(cn[:], cnc[:], AF.Sqrt, bias=beps[:])
            pl = small.tile([128, M_TILES], f32, tag="pl")
            nl = small.tile([128, M_TILES], f32, tag="nl")
            nc.scalar.activation(pl[:], fp[:], AF.Relu, bias=bpos[:])
            nc.scalar.activation(nl[:], cn[:], AF.Relu, bias=bneg[:],
                                 scale=-1.0)
            nc.vector.tensor_reduce(out=accT[:, 0:1], in_=pl[:], op=A.add,
                                    axis=X)
            nc.vector.tensor_reduce(out=accT[:, 1:2], in_=nl[:], op=A.add,
                                    axis=X)
            pso = pf_p.tile([2, 1], f32, tag="pf")
            nc.tensor.matmul(out=pso[:], lhsT=accT[:], rhs=ones128[:],
                             start=True, stop=True)
            res_sb = small.tile([2, 1], f32, tag="res")
            nc.scalar.copy(res_sb[:], pso[:])
            nc.sync.dma_start(out_d[:], res_sb[:])

    nc.compile()
    return nc


def make_in_maps(src_pcd, tgt_pcd, src_feats, tgt_feats, correspondence,
                 rot, trans):
    """Host-side gather/shard/layout (indexing/transpose + constant rows)."""
    ci = np.asarray(correspondence[:, 0]).astype(np.int64)
    cj = np.asarray(correspondence[:, 1]).astype(np.int64)
    src_pcd = np.asarray(src_pcd, np.float32)
    tgt_pcd = np.asarray(tgt_pcd, np.float32)
    src_feats = np.asarray(src_feats, np.float32)
    tgt_feats = np.asarray(tgt_feats, np.float32)
    SC1 = np.float32(S) * np.float32(C1)
    SC2 = np.float32(S) * np.float32(C2)

    # rhs feats ext: [tf | 1 | tf^2(dev)]
    tgtF = np.zeros((64, N_SEL), np.float32)
    tgtF[0:32] = tgt_feats[cj].T
    tgtF[32:64] = 1.0
    # rhs pts ext: [tp | -S | 1 | tp^2(dev)]
    tgtP = np.zeros((K_P, N_SEL), np.float32)
    tgtP[0:3] = tgt_pcd[cj].T
    tgtP[3:6] = -np.float32(S)
    tgtP[6] = 1.0

    # lhs feats ext: [sf(dev: *-2) | sf^2(dev) | 1]
    srcF = np.zeros((K_F, N_SEL), np.float32)
    srcF[0:32] = src_feats[ci].T
    srcF[64:96] = 1.0
    # lhs pts ext: [sp(dev: rot+2S) | sp'^2(dev) | SCx | -S]
    srcN = np.zeros((K_P, N_SEL), np.float32)
    srcN[0:3] = src_pcd[ci].T
    srcN[6] = SC2
    srcN[7:10] = -np.float32(S)
    srcO = srcN.copy()
    srcO[6] = SC1

    rtt = np.zeros((3, 4), np.float32)
    rtt[:, 0:3] = np.asarray(rot, np.float32).T
    rtt[:, 3] = np.asarray(trans, np.float32)[:, 0]

    in_maps = []
    for c in range(N_CORES):
        sl = slice(c * ROWS_PER_CORE, (c + 1) * ROWS_PER_CORE)
        in_maps.append({
            "srcF": np.ascontiguousarray(srcF[:, sl]),
            "srcN": np.ascontiguousarray(srcN[:, sl]),
            "srcO": np.ascontiguousarray(srcO[:, sl]),
            "tgtF": tgtF,
            "tgtP": tgtP,
            "rtt": rtt,
        })
    return in_maps


def combine_outputs(results):
    """Host-side unshard: sum per-core partial sums, divide by N."""
    tot = np.zeros(2, np.float32)
    for r in results:
        tot += r["out"][:, 0].astype(np.float32)
    loss = np.float32(tot[0] / np.float32(N_SEL) + tot[1] / np.float32(N_SEL))
    return np.float32(loss)


def kernel(src_pcd, tgt_pcd, src_feats, tgt_feats, correspondence, rot,
           trans):
    from concourse import bass_utils

    key = ("prog", 1, KERNEL_CFG["dve_tiles"], KERNEL_CFG["mm"],
           KERNEL_CFG["wide"])
    if key not in _PROGRAM_CACHE:
        _PROGRAM_CACHE[key] = build_program(
            repeat=1, dve_tiles=KERNEL_CFG["dve_tiles"], mm=KERNEL_CFG["mm"],
            wide=KERNEL_CFG["wide"])
    nc = _PROGRAM_CACHE[key]
    in_maps = make_in_maps(src_pcd, tgt_pcd, src_feats, tgt_feats,
                           correspondence, rot, trans)
    res = bass_utils.run_bass_kernel_spmd(nc, in_maps,
                                          core_ids=list(range(N_CORES)))
    return combine_outputs(res.results)


# revision 34
# speedup vs baseline: 1.4581x; 1.4581x over previous
"""Trainium2 Bass kernel for nn_HardestContrastiveLoss.

Strategy (1D row-parallel cdist, per sharding hint):
  - Host: gather the selected correspondences (pure indexing/layout), build
    transposed operand blocks with constant padding rows, shard 8192 selected
    rows as 1024 rows/core.
  - Device (per core, identical program, different data):
      * prep: rigid-transform gathered src points, square feats/points; all
        norm and threshold terms ride as extra contraction rows of the main
        GEMMs (matmul cost is streaming-bound, independent of K):
        psf  = [-2sf | sf^2 | 1] . [tf | 1 | tf^2]        K=96 -> feats_d2
        pneg = [2S sp'|sp'^2|SC2|-S] . [tp|-S|1|tp^2]     K=10 -> -S(pts_d2-C2)
        ppos = same with SC1                              K=10 -> -S(pts_d2-C1)
      * hardest-positive per row:  max_j min(psf, ppos)   (clamped at 0)
        closest-negative per row:  min_j max(psf, pneg)   (init BIG)
        ONE fused DVE tensor_tensor_reduce per (tile, path) for most n-tiles;
        the rest accumulate elementwise on the Pool engine (Pool cannot
        row-reduce) with one DVE reduce per row-block.
      * all matmuls in float32r (1 PE cycle/row at free-size 512 vs 4 for
        fp32); sqrt/thresholds deferred to the [128, 8] tail.
  - Host: sum the 8 per-core [2,1] partials, divide by N (the "all-reduce").
"""

import numpy as np

N_SEL = 8192
N_CORES = 8
ROWS_PER_CORE = N_SEL // N_CORES  # 1024
M_TILES = ROWS_PER_CORE // 128  # 8
NT = 512
N_TILES = N_SEL // NT  # 16
K_F = 96  # feats ext contraction: 32 feats + 32 row-norm + 32 col-norm
K_P = 10  # pts ext contraction: 3 pts + 3 row-norm + 1 const + 3 col-norm

EPS = 1e-7
POS_RADIUS = 0.0375
NEG_RADIUS = 0.1
POS_THRESH = 0.1
NEG_THRESH = 1.4
C1 = float(np.float32(POS_RADIUS**2 - EPS))  # pos: pd2 < C1
C2 = float(np.float32(NEG_RADIUS**2 - EPS))  # neg: pd2 > C2
S = 1.0e13
BIG = 100000.0

_PROGRAM_CACHE: dict = {}
KERNEL_CFG = {"dve_tiles": 16, "mm": "f32r", "wide": False}


def _register_fused_ops():
    """Register two custom DVE ops (select + row-reduce fused in one
    instruction): pos = max-reduce of min(in0, in1), neg = min-reduce of
    max(in0, in1); accum seeded from s0. Idempotent; appended after the
    stock OPS so existing rows are untouched."""
    import re
    from concourse import dve_ops
    from concourse.dve_spec import Spec, Src0, Src1, C0, maxx, minn

    def _ref(ew_fn, red_fn, seed_fn):
        def _r(in0, in1, c0, c1, c2):
            b = ew_fn(in0.astype(np.float32), in1.astype(np.float32))
            r = red_fn(b.reshape(b.shape[0], -1), axis=-1, keepdims=True)
            return b, seed_fn(np.float32(c0), r)
        return _r

    refs = {
        "SELMIN_REDMAX_ANT": _ref(np.minimum, np.max, np.maximum),
        "SELMAX_REDMIN_ANT": _ref(np.maximum, np.min, np.minimum),
    }

    def reg(name, body, accum):
        if name in dve_ops._SUB_OPCODE_FOR_NAME:
            for op in dve_ops.OPS:
                if op.name == name:
                    return op
        op = dve_ops.DveOp(name, Spec(body=body, accum=accum, accum_init=C0,
                                      reference=refs[name]),
                           subdim=False, uops_sha={})
        dve_ops.OPS.append(op)
        dve_ops.CUSTOM_DVE_SPECS[name] = op.spec
        dve_ops._SUB_OPCODE_FOR_NAME[name] = (
            dve_ops._CUSTOM_DVE_ROW_BASE + len(dve_ops.OPS) - 1)
        for ver in ("v3", "v4"):
            try:
                op.compile(ver)
            except ValueError as e:
                m = re.search(r'"' + ver + r'"\]="([0-9a-f]+)"', str(e))
                if m:
                    op.uops_sha[ver] = m.group(1)
                    op.compile(ver)
        return op

    pos = reg("SELMIN_REDMAX_ANT", minn(Src0, Src1), maxx)
    neg = reg("SELMAX_REDMIN_ANT", maxx(Src0, Src1), minn)
    return pos, neg


def build_program(repeat: int = 1, dve_tiles: int = 16, mm: str = "f32r",
                  fused: bool = True, wide: bool = False, **_legacy):
    """Build the Bass program (one NeuronCore, run SPMD on 8).

    dve_tiles: how many of the 16 n-tiles use the fused DVE
    tensor_tensor_reduce for both paths; the rest run elementwise min/max
    accumulation on the Pool engine. mm: matmul input dtype ("f32r"|"f32").
    """
    import concourse.bacc as bacc
    import concourse.mybir as mybir
    import concourse.tile as tile

    f32 = mybir.dt.float32
    f32r = mybir.dt.float32r
    A = mybir.AluOpType
    AF = mybir.ActivationFunctionType
    X = mybir.AxisListType.X

    mmdt = f32r if mm == "f32r" else f32

    def ecast(ap):
        # engine-side (non-PE) view of an fp32r tile
        return ap.bitcast(f32) if mm == "f32r" else ap

    POS_OP, NEG_OP = _register_fused_ops()

    # n-tile -> engine assignment: interleave pool tiles among dve tiles so
    # both engines stay fed. pool_set = odd tiles first.
    n_pool = N_TILES - dve_tiles
    pool_set = set()
    n = 1
    while len(pool_set) < n_pool:
        pool_set.add(n)
        n = n + 2 if n + 2 < N_TILES else (0 if 0 not in pool_set else n + 1)
    pool_order = sorted(pool_set)

    nc = bacc.Bacc("TRN2", target_bir_lowering=False, debug=False,
                   num_devices=N_CORES)
    srcF_d = nc.dram_tensor("srcF", [K_F, ROWS_PER_CORE], mmdt,
                            kind="ExternalInput").ap()
    srcN_d = nc.dram_tensor("srcN", [K_P, ROWS_PER_CORE], mmdt,
                            kind="ExternalInput").ap()
    srcO_d = nc.dram_tensor("srcO", [K_P, ROWS_PER_CORE], mmdt,
                            kind="ExternalInput").ap()
    tgtF_d = nc.dram_tensor("tgtF", [64, N_SEL], mmdt,
                            kind="ExternalInput").ap()
    tgtP_d = nc.dram_tensor("tgtP", [K_P, N_SEL], mmdt,
                            kind="ExternalInput").ap()
    rtt_d = nc.dram_tensor("rtt", [3, 4], mmdt, kind="ExternalInput").ap()
    out_d = nc.dram_tensor("out", [2, 1], f32, kind="ExternalOutput").ap()

    with tile.TileContext(nc) as tc:
        with (
            tc.tile_pool(name="big", bufs=1) as big,
            tc.tile_pool(name="scr", bufs=2) as scr_p,
            tc.tile_pool(name="pscr", bufs=2) as pscr_p,
            tc.tile_pool(name="acc", bufs=2) as acc_p,
            tc.tile_pool(name="red", bufs=2) as red_p,
            tc.tile_pool(name="small", bufs=4) as small,
            tc.tile_pool(name="pf", bufs=2, space="PSUM") as pf_p,
            tc.tile_pool(name="pn", bufs=(1 if wide else 2),
                         space="PSUM") as pn_p,
            tc.tile_pool(name="pp", bufs=(1 if wide else 2),
                         space="PSUM") as pp_p,
        ):
            rhsF = big.tile([K_F, N_SEL], mmdt, tag="rhsF")
            rhsP = big.tile([K_P, N_SEL], mmdt, tag="rhsP")
            lhsF = big.tile([K_F, ROWS_PER_CORE], mmdt, tag="lhsF")
            lhsN = big.tile([K_P, ROWS_PER_CORE], mmdt, tag="lhsN")
            lhsP = big.tile([K_P, ROWS_PER_CORE], mmdt, tag="lhsP")
            rtt_sb = big.tile([3, 4], mmdt, tag="rtt")
            praw = big.tile([3, ROWS_PER_CORE], mmdt, tag="praw")
            prot = big.tile([3, ROWS_PER_CORE], f32, tag="prot")
            sq32 = big.tile([32, ROWS_PER_CORE], mmdt, tag="sq32")
            sqSP = big.tile([3, ROWS_PER_CORE], mmdt, tag="sqSP")
            sqt32 = big.tile([32, N_SEL], mmdt, tag="sqt32")
            sqP = big.tile([3, N_SEL], mmdt, tag="sqP")
            ones128 = big.tile([128, 1], f32, tag="ones128")
            fp2all = big.tile([128, M_TILES], f32, tag="fp2all")
            cn2all = big.tile([128, M_TILES], f32, tag="cn2all")
            accT = big.tile([128, 2], f32, tag="accT")
            beps = big.tile([128, 1], f32, tag="beps")
            bpos = big.tile([128, 1], f32, tag="bpos")
            bneg = big.tile([128, 1], f32, tag="bneg")

            nc.sync.dma_start(rhsF[0:64, :], tgtF_d[:])
            nc.sync.dma_start(rhsP[:], tgtP_d[:])
            nc.sync.dma_start(lhsF[:], srcF_d[:])
            nc.sync.dma_start(lhsN[:], srcN_d[:])
            nc.sync.dma_start(lhsP[:], srcO_d[:])
            nc.sync.dma_start(rtt_sb[:], rtt_d[:])
            nc.sync.dma_start(praw[:], srcN_d[0:3, :])
            nc.gpsimd.memset(ones128[:], 1.0)
            nc.gpsimd.memset(beps[:], EPS)
            nc.gpsimd.memset(bpos[:], -POS_THRESH)
            nc.gpsimd.memset(bneg[:], NEG_THRESH)

            # ---- squares of raw feats (before -2 scaling) ----
            nc.scalar.activation(sq32[:], ecast(lhsF[0:32, :]), AF.Square)
            nc.scalar.activation(sqt32[:], ecast(rhsF[0:32, :]), AF.Square)
            nc.scalar.activation(sqP[:], ecast(rhsP[0:3, :]), AF.Square)
            nc.sync.dma_start(lhsF[32:64, :], sq32[:])
            nc.sync.dma_start(rhsF[64:96, :], sqt32[:])
            nc.sync.dma_start(rhsP[7:10, :], sqP[:])
            nc.scalar.activation(lhsF[0:32, :], ecast(lhsF[0:32, :]),
                                 AF.Copy, bias=0.0, scale=-2.0)

            # ---- rigid transform: prot = R @ sp + t ----
            for ch in range(ROWS_PER_CORE // NT):
                sl = slice(ch * NT, (ch + 1) * NT)
                pch = pf_p.tile([3, NT], f32, tag="pf")
                nc.tensor.matmul(out=pch[:], lhsT=rtt_sb[0:3, 0:3],
                                 rhs=praw[:, sl], start=True,
                                 stop=True)
                nc.vector.tensor_scalar(out=prot[:, sl], in0=pch[:],
                                        scalar1=ecast(rtt_sb[0:3, 3:4]),
                                        scalar2=None, op0=A.add)
            nc.scalar.activation(sqSP[:], prot[:], AF.Square)
            nc.scalar.activation(lhsN[0:3, :], prot[:], AF.Copy,
                                 bias=0.0, scale=2.0 * S)
            nc.scalar.activation(lhsP[0:3, :], prot[:], AF.Copy,
                                 bias=0.0, scale=2.0 * S)
            nc.sync.dma_start(lhsN[3:6, :], sqSP[:])
            nc.sync.dma_start(lhsP[3:6, :], sqSP[:])

            WNT = 2 * NT if wide else NT
            WN_TILES = N_SEL // WNT
            pos_rA = big.tile([128, M_TILES * WN_TILES], f32, tag="pos_rA")
            neg_rA = big.tile([128, M_TILES * WN_TILES], f32, tag="neg_rA")

            def main_loop(_iv=None):
                for m in range(M_TILES):
                    msl = slice(m * 128, (m + 1) * 128)
                    pos_r = pos_rA[:, m * WN_TILES:(m + 1) * WN_TILES]
                    neg_r = neg_rA[:, m * WN_TILES:(m + 1) * WN_TILES]
                    for n in range(WN_TILES):
                        psf = pf_p.tile([128, WNT], f32, tag="pf")
                        png = pn_p.tile([128, WNT], f32, tag="pn")
                        pps = pp_p.tile([128, WNT], f32, tag="pp")
                        for g in range(WNT // NT):
                            nsl = slice(n * WNT + g * NT,
                                        n * WNT + (g + 1) * NT)
                            gsl = slice(g * NT, (g + 1) * NT)
                            nc.tensor.matmul(out=psf[:, gsl],
                                             lhsT=lhsF[:, msl],
                                             rhs=rhsF[:, nsl],
                                             start=True, stop=True)
                            nc.tensor.matmul(out=png[:, gsl],
                                             lhsT=lhsN[:, msl],
                                             rhs=rhsP[:, nsl],
                                             start=True, stop=True)
                            nc.tensor.matmul(out=pps[:, gsl],
                                             lhsT=lhsP[:, msl],
                                             rhs=rhsP[:, nsl],
                                             start=True, stop=True)
                        fsb = scr_p.tile([128, WNT], f32, tag="fsb")
                        nc.scalar.copy(fsb[:], psf[:])
                        s1 = scr_p.tile([128, WNT], f32, tag="s1")
                        s2 = scr_p.tile([128, WNT], f32, tag="s2")
                        if fused:
                            nc.vector._custom_dve(
                                POS_OP, out=s1[:], in0=fsb[:], in1=pps[:],
                                s0=0.0, accum_out=pos_r[:, n:n + 1])
                            nc.vector._custom_dve(
                                NEG_OP, out=s2[:], in0=fsb[:], in1=png[:],
                                s0=BIG, accum_out=neg_r[:, n:n + 1])
                        else:
                            nc.vector.tensor_tensor(out=s1[:], in0=fsb[:],
                                                    in1=pps[:], op=A.min)
                            nc.vector.tensor_tensor(out=s2[:], in0=fsb[:],
                                                    in1=png[:], op=A.max)
                            nc.vector.tensor_reduce(
                                out=pos_r[:, n:n + 1], in_=s1[:], op=A.max,
                                axis=X)
                            nc.vector.tensor_reduce(
                                out=neg_r[:, n:n + 1], in_=s2[:], op=A.min,
                                axis=X)
            if repeat <= 4:
                for _ in range(repeat):
                    main_loop()
            else:
                with tc.For_i(0, repeat, 1) as iv:
                    main_loop(iv)

            # ---- tail: clamp / sqrt / relu thresholds / partition sums ----
            for m in range(M_TILES):
                nc.vector.tensor_reduce(
                    out=fp2all[:, m:m + 1],
                    in_=pos_rA[:, m * WN_TILES:(m + 1) * WN_TILES],
                    op=A.max, axis=X)
                nc.vector.tensor_reduce(
                    out=cn2all[:, m:m + 1],
                    in_=neg_rA[:, m * WN_TILES:(m + 1) * WN_TILES],
                    op=A.min, axis=X)
            fpc = small.tile([128, M_TILES], f32, tag="fpc")
            cnc = small.tile([128, M_TILES], f32, tag="cnc")
            nc.scalar.activation(fpc[:], fp2all[:], AF.Relu)
            nc.scalar.activation(cnc[:], cn2all[:], AF.Relu)
            fp = small.tile([128, M_TILES], f32, tag="fp")
            cn = small.tile([128, M_TILES], f32, tag="cn")
            nc.scalar.activation(fp[:], fpc[:], AF.Sqrt, bias=beps[:])
            nc.scalar.activation(cn[:], cnc[:], AF.Sqrt, bias=beps[:])
            pl = small.tile([128, M_TILES], f32, tag="pl")
            nl = small.tile([128, M_TILES], f32, tag="nl")
            nc.scalar.activation(pl[:], fp[:], AF.Relu, bias=bpos[:])
            nc.scalar.activation(nl[:], cn[:], AF.Relu, bias=bneg[:],
                                 scale=-1.0)
            nc.vector.tensor_reduce(out=accT[:, 0:1], in_=pl[:], op=A.add,
                                    axis=X)
            nc.vector.tensor_reduce(out=accT[:, 1:2], in_=nl[:], op=A.add,
                                    axis=X)
            pso = pf_p.tile([2, 1], f32, tag="pf")
            nc.tensor.matmul(out=pso[:], lhsT=accT[:], rhs=ones128[:],
                             start=True, stop=True)
            res_sb = small.tile([2, 1], f32, tag="res")
            nc.scalar.copy(res_sb[:], pso[:])
            nc.sync.dma_start(out_d[:], res_sb[:])

    nc.compile()
    return nc


def make_in_maps(src_pcd, tgt_pcd, src_feats, tgt_feats, correspondence,
                 rot, trans):
    """Host-side gather/shard/layout (indexing/transpose + constant rows)."""
    ci = np.asarray(correspondence[:, 0]).astype(np.int64)
    cj = np.asarray(correspondence[:, 1]).astype(np.int64)
    src_pcd = np.asarray(src_pcd, np.float32)
    tgt_pcd = np.asarray(tgt_pcd, np.float32)
    src_feats = np.asarray(src_feats, np.float32)
    tgt_feats = np.asarray(tgt_feats, np.float32)
    SC1 = np.float32(S) * np.float32(C1)
    SC2 = np.float32(S) * np.float32(C2)

    # rhs feats ext: [tf | 1 | tf^2(dev)]
    tgtF = np.zeros((64, N_SEL), np.float32)
    tgtF[0:32] = tgt_feats[cj].T
    tgtF[32:64] = 1.0
    # rhs pts ext: [tp | -S | 1 | tp^2(dev)]
    tgtP = np.zeros((K_P, N_SEL), np.float32)
    tgtP[0:3] = tgt_pcd[cj].T
    tgtP[3:6] = -np.float32(S)
    tgtP[6] = 1.0

    # lhs feats ext: [sf(dev: *-2) | sf^2(dev) | 1]
    srcF = np.zeros((K_F, N_SEL), np.float32)
    srcF[0:32] = src_feats[ci].T
    srcF[64:96] = 1.0
    # lhs pts ext: [sp(dev: rot+2S) | sp'^2(dev) | SCx | -S]
    srcN = np.zeros((K_P, N_SEL), np.float32)
    srcN[0:3] = src_pcd[ci].T
    srcN[6] = SC2
    srcN[7:10] = -np.float32(S)
    srcO = srcN.copy()
    srcO[6] = SC1

    rtt = np.zeros((3, 4), np.float32)
    rtt[:, 0:3] = np.asarray(rot, np.float32).T
    rtt[:, 3] = np.asarray(trans, np.float32)[:, 0]

    in_maps = []
    for c in range(N_CORES):
        sl = slice(c * ROWS_PER_CORE, (c + 1) * ROWS_PER_CORE)
        in_maps.append({
            "srcF": np.ascontiguousarray(srcF[:, sl]),
            "srcN": np.ascontiguousarray(srcN[:, sl]),
            "srcO": np.ascontiguousarray(srcO[:, sl]),
            "tgtF": tgtF,
            "tgtP": tgtP,
            "rtt": rtt,
        })
    return in_maps


def combine_outputs(results):
    """Host-side unshard: sum per-core partial sums, divide by N."""
    tot = np.zeros(2, np.float32)
    for r in results:
        tot += r["out"][:, 0].astype(np.float32)
    loss = np.float32(tot[0] / np.float32(N_SEL) + tot[1] / np.float32(N_SEL))
    return np.float32(loss)


def kernel(src_pcd, tgt_pcd, src_feats, tgt_feats, correspondence, rot,
           trans):
    from concourse import bass_utils

    key = ("prog", 1, KERNEL_CFG["dve_tiles"], KERNEL_CFG["mm"],
           KERNEL_CFG["wide"])
    if key not in _PROGRAM_CACHE:
        _PROGRAM_CACHE[key] = build_program(
            repeat=1, dve_tiles=KERNEL_CFG["dve_tiles"], mm=KERNEL_CFG["mm"],
            wide=KERNEL_CFG["wide"])
    nc = _PROGRAM_CACHE[key]
    in_maps = make_in_maps(src_pcd, tgt_pcd, src_feats, tgt_feats,
                           correspondence, rot, trans)
    res = bass_utils.run_bass_kernel_spmd(nc, in_maps,
                                          core_ids=list(range(N_CORES)))
    return combine_outputs(res.results)
